# revision 1
# baseline (speedup 1.0000x reference)
"""Bass/Trainium2 kernel for nn_KernelAMController (retrieval_knn).

Math: out(b,:) = -sum_g w_eff(b,g)*adj[tb(b),g,:] / (sum_g w_eff(b,g) + eps)
with w_eff(b,g) = exp(-2*||x_b - p_g||^2) * (counts[tb(b),g] > 0).

Restructured as two matmuls per 512-sample group (data-parallel over B on 8
cores, grid buffers replicated):
  mm1: W^T(g,b) = exp(Pa^T @ Xa)  — augmented split-bf16 matmul gives the full
       exponent -2*||x-p||^2 directly (K=15: hi*hi, hi*lo, lo*hi blocks).
  mm2: Y^T(m,b) = sum_g Ct(g,m) * W^T(g,b) accumulated over 20 g-chunks in
       PSUM, where Ct columns m = d*20+k hold [mask*adj_x | mask*adj_y | mask]
       per time-bin k.
  Selection: one-hot over the 20 bins built from strict > comparisons against
       t_edges (searchsorted-left semantics), applied elementwise to Y^T, then
       reduced over k via a tiny +/-1 block matrix matmul (numerators negated
       there for free). Small PE transposes flip (3,B) -> (B,3) for the final
       per-sample divide.
"""
import numpy as np
import ml_dtypes

import concourse.bass as bass
import concourse.tile as tile
from concourse import mybir, bacc
from concourse.bass_utils import run_bass_kernel_spmd

F32 = mybir.dt.float32
BF16 = mybir.dt.bfloat16
BF16_NP = ml_dtypes.bfloat16

B = 32768
G = 2500
GP = 2560          # padded grid (20 chunks of 128)
NCHUNK = 20
NBINS = 20
NCORES = 8
BC = B // NCORES   # 4096 samples per core
NGRP = 8           # groups per core
BG = BC // NGRP    # 512 samples per group
EPS = 1e-10

_CACHE = {}


def _build_nc():
    nc = bacc.Bacc("TRN2", target_bir_lowering=False)
    x_d = nc.dram_tensor("xstage", [2, BC], F32, kind="ExternalInput")
    on_d = nc.dram_tensor("ones3", [3, BG], BF16, kind="ExternalInput")
    t_d = nc.dram_tensor("trep", [60, BC], F32, kind="ExternalInput")
    pa_d = nc.dram_tensor("pa", [15, GP], BF16, kind="ExternalInput")
    ct_d = nc.dram_tensor("ct", [128, NCHUNK * 64], BF16, kind="ExternalInput")
    ea_d = nc.dram_tensor("ea", [60, 1], F32, kind="ExternalInput")
    eb_d = nc.dram_tensor("eb", [60, 1], F32, kind="ExternalInput")
    bn_d = nc.dram_tensor("bones", [60, 3], BF16, kind="ExternalInput")
    id_d = nc.dram_tensor("ident", [3, 3], F32, kind="ExternalInput")
    o_d = nc.dram_tensor("o", [NGRP, 128, 8], F32, kind="ExternalOutput")

    gt = mybir.AluOpType.is_gt
    with tile.TileContext(nc) as tc:
        with (
            tc.tile_pool(name="consts", bufs=1) as consts,
            tc.tile_pool(name="xin", bufs=2) as xin,
            tc.tile_pool(name="tin", bufs=2) as tin,
            tc.tile_pool(name="xa", bufs=2) as xap,
            tc.tile_pool(name="oh", bufs=2) as oh,
            tc.tile_pool(name="wt", bufs=3) as wtp,
            tc.tile_pool(name="r3", bufs=2) as r3p,
            tc.tile_pool(name="ep", bufs=2) as ep,
            tc.tile_pool(name="pw", bufs=2, space="PSUM") as pwp,
            tc.tile_pool(name="py", bufs=2, space="PSUM") as pyp,
            tc.tile_pool(name="pr", bufs=1, space="PSUM") as prp,
            tc.tile_pool(name="prt", bufs=1, space="PSUM") as prtp,
        ):
            pa_sb = consts.tile([15, GP], BF16)
            nc.sync.dma_start(out=pa_sb[:], in_=pa_d[:])
            ct_sb = consts.tile([128, NCHUNK * 64], BF16)
            nc.sync.dma_start(out=ct_sb[:], in_=ct_d[:])
            ea_sb = consts.tile([60, 1], F32)
            nc.sync.dma_start(out=ea_sb[:], in_=ea_d[:])
            eb_sb = consts.tile([60, 1], F32)
            nc.sync.dma_start(out=eb_sb[:], in_=eb_d[:])
            bn_sb = consts.tile([60, 3], BF16)
            nc.sync.dma_start(out=bn_sb[:], in_=bn_d[:])
            id_sb = consts.tile([3, 3], F32)
            nc.sync.dma_start(out=id_sb[:], in_=id_d[:])
            on_sb = consts.tile([3, BG], BF16)
            nc.sync.dma_start(out=on_sb[:], in_=on_d[:])

            for g in range(NGRP):
                s0 = g * BG
                xf = xin.tile([2, BG], F32)
                nc.sync.dma_start(out=xf[:], in_=x_d[:, s0:s0 + BG])
                tr = tin.tile([60, BG], F32)
                nc.sync.dma_start(out=tr[:], in_=t_d[:, s0:s0 + BG])

                # hi/lo bf16 split of [x0, x1] and [x0^2, x1^2] on partitions 0-1,
                # then DMA-assemble the K=15 moving operand (DMA moves across
                # partitions; compute engines are partition-locked and need
                # 32-aligned bases).
                sq = xap.tile([2, BG], F32, tag="sq")
                nc.vector.tensor_mul(sq[:], xf[:], xf[:])
                xh = xap.tile([2, BG], BF16, tag="xh")
                nc.vector.tensor_copy(xh[:], xf[:])
                xl = xap.tile([2, BG], BF16, tag="xl")
                nc.vector.tensor_sub(xl[:], xf[:], xh[:])
                sqh = xap.tile([2, BG], BF16, tag="sqh")
                nc.vector.tensor_copy(sqh[:], sq[:])
                sql = xap.tile([2, BG], BF16, tag="sql")
                nc.vector.tensor_sub(sql[:], sq[:], sqh[:])
                xa = xap.tile([15, BG], BF16, tag="xa")
                nc.sync.dma_start(out=xa[0:2], in_=xh[:])
                nc.sync.dma_start(out=xa[2:4], in_=sqh[:])
                nc.sync.dma_start(out=xa[5:7], in_=xl[:])
                nc.sync.dma_start(out=xa[7:9], in_=sql[:])
                nc.sync.dma_start(out=xa[10:12], in_=xh[:])
                nc.sync.dma_start(out=xa[12:14], in_=sqh[:])
                xa_c = xa.rearrange("(a b) n -> a b n", b=5)[:, 4, :]
                nc.sync.dma_start(out=xa_c, in_=on_sb[:])

                # one-hot over bins (strict >, searchsorted-left semantics)
                sa = oh.tile([60, BG], F32, tag="sa")
                nc.vector.tensor_scalar(sa[:], tr[:], ea_sb[:], None, gt)
                sb_ = oh.tile([60, BG], F32, tag="sb")
                nc.vector.tensor_scalar(sb_[:], tr[:], eb_sb[:], None, gt)
                o3 = oh.tile([60, BG], BF16, tag="o3")
                nc.vector.tensor_sub(o3[:], sa[:], sb_[:])

                # Software-pipelined by one chunk-pair: pair q+1's mm1s are
                # emitted before pair q's mm2s so the in-order PE queue has
                # independent work while ScalarE computes exp(pair q).
                py = pyp.tile([64, BG], F32)
                pend = None
                for q in range(NCHUNK // 2):
                    pw = pwp.tile([128, 2, BG], F32)
                    for j in (0, 1):
                        c = 2 * q + j
                        nc.tensor.matmul(
                            pw[:, j, :], lhsT=pa_sb[:, c * 128:(c + 1) * 128],
                            rhs=xa[:], start=True, stop=True)
                    wt = wtp.tile([128, 2, BG], BF16)
                    nc.scalar.activation(wt[:], pw[:],
                                         mybir.ActivationFunctionType.Exp)
                    if pend is not None:
                        wp, qp = pend
                        for j in (0, 1):
                            c = 2 * qp + j
                            nc.tensor.matmul(
                                py[:], lhsT=ct_sb[:, c * 64:(c + 1) * 64],
                                rhs=wp[:, j, :], start=(c == 0), stop=False)
                    pend = (wt, q)
                wp, qp = pend
                for j in (0, 1):
                    c = 2 * qp + j
                    nc.tensor.matmul(
                        py[:], lhsT=ct_sb[:, c * 64:(c + 1) * 64],
                        rhs=wp[:, j, :], start=False, stop=(c == NCHUNK - 1))

                r3 = r3p.tile([60, BG], BF16)
                nc.vector.tensor_mul(r3[:], py[0:60, :], o3[:])
                pr = prp.tile([3, BG], F32)
                nc.tensor.matmul(pr[:], lhsT=bn_sb[:], rhs=r3[:], start=True,
                                 stop=True)
                rsb = ep.tile([3, BG], F32, tag="rsb")
                nc.vector.tensor_copy(rsb[:], pr[:])
                prt = prtp.tile([128, 12], F32)
                for s in range(4):
                    nc.tensor.transpose(prt[:, s * 3:(s + 1) * 3],
                                        rsb[:, s * 128:(s + 1) * 128], id_sb[:])
                prt3 = prt.rearrange("p (s c) -> p s c", c=3)
                den = ep.tile([128, 4], F32, tag="den")
                nc.vector.tensor_scalar(den[:], prt3[:, :, 2], EPS, None,
                                        mybir.AluOpType.add)
                rec = ep.tile([128, 4], F32, tag="rec")
                nc.vector.reciprocal(rec[:], den[:])
                ot = ep.tile([128, 8], F32, tag="ot")
                ot2 = ot.rearrange("p (s c) -> p s c", c=2)
                nc.vector.tensor_mul(ot2[:, :, 0], prt3[:, :, 0], rec[:])
                nc.vector.tensor_mul(ot2[:, :, 1], prt3[:, :, 1], rec[:])
                nc.sync.dma_start(out=o_d[g], in_=ot[:])
    nc.compile()
    return nc


def _host_prep(t, x, grid_points, grid_adjoints, t_edges, grid_counts):
    t = np.asarray(t, np.float32).reshape(B)
    x = np.asarray(x, np.float32)
    gp = np.asarray(grid_points, np.float32)
    adj = np.asarray(grid_adjoints, np.float32)
    te = np.asarray(t_edges, np.float32)
    cnt = np.asarray(grid_counts)

    mask = (cnt > 0).astype(np.float32)               # (20, G)
    ct = np.zeros((GP, 64), np.float32)
    ct[:G, 0:20] = (mask * adj[:, :, 0]).T
    ct[:G, 20:40] = (mask * adj[:, :, 1]).T
    ct[:G, 40:60] = mask.T
    ct_dram = np.ascontiguousarray(
        ct.reshape(NCHUNK, 128, 64).transpose(1, 0, 2).reshape(128, NCHUNK * 64)
    ).astype(BF16_NP)

    p5 = np.zeros((5, GP), np.float32)
    p5[0, :G] = 4.0 * gp[:, 0]
    p5[1, :G] = 4.0 * gp[:, 1]
    p5[2, :G] = -2.0
    p5[3, :G] = -2.0
    p5[4, :G] = -2.0 * (gp[:, 0] ** 2 + gp[:, 1] ** 2)
    p5[4, G:] = -1e30
    ph = p5.astype(BF16_NP)
    pl = (p5 - ph.astype(np.float32)).astype(BF16_NP)
    pa = np.concatenate([ph, ph, pl], axis=0)          # (15, GP) bf16

    ea = np.concatenate([[-1.0], te[1:20]]).astype(np.float32)   # (20,)
    eb = te[1:21].astype(np.float32)                              # (20,)
    ea3 = np.tile(ea, 3).reshape(60, 1)
    eb3 = np.tile(eb, 3).reshape(60, 1)

    bones = np.zeros((60, 3), np.float32)
    for d in range(3):
        bones[d * 20:(d + 1) * 20, d] = 1.0 if d == 2 else -1.0
    bones = bones.astype(BF16_NP)
    ident = np.eye(3, dtype=np.float32)

    ones3 = np.zeros((3, BG), np.float32)
    ones3[0] = 1.0
    ones3[2] = 1.0
    ones3 = ones3.astype(BF16_NP)

    in_maps = []
    for i in range(NCORES):
        xs = x[i * BC:(i + 1) * BC]                    # (BC, 2)
        ts = t[i * BC:(i + 1) * BC]                    # (BC,)
        xstage = np.ascontiguousarray(xs.T)            # (2, BC)
        trep = np.ascontiguousarray(np.broadcast_to(ts, (60, BC)))
        in_maps.append({
            "xstage": xstage, "trep": trep, "pa": pa, "ct": ct_dram,
            "ea": ea3, "eb": eb3, "bones": bones, "ident": ident,
            "ones3": ones3,
        })
    return in_maps


def kernel(t, x, grid_points, grid_adjoints, t_edges, grid_counts,
           trace=False, tmpdir=None):
    if "nc" not in _CACHE:
        _CACHE["nc"] = _build_nc()
    nc = _CACHE["nc"]
    in_maps = _host_prep(t, x, grid_points, grid_adjoints, t_edges, grid_counts)
    res = run_bass_kernel_spmd(nc, in_maps, core_ids=list(range(NCORES)),
                               trace=trace, tmpdir=tmpdir)
    _CACHE["last_result"] = res
    out = np.empty((B, 2), np.float32)
    for i in range(NCORES):
        raw = res.results[i]["o"].reshape(NGRP, 128, 4, 2)
        out[i * BC:(i + 1) * BC] = raw.transpose(0, 2, 1, 3).reshape(BC, 2)
    return out



# revision 2
# speedup vs baseline: 1.0308x; 1.0308x over previous
"""Bass/Trainium2 kernel v3 for nn_KernelAMController (retrieval_knn).

Math: out(b,:) = -sum_g w_eff(b,g)*adj[tb(b),g,:] / (sum_g w_eff(b,g) + eps)
with w_eff(b,g) = exp(-2*||x_b - p_g||^2) * (counts[tb(b),g] > 0).

Design:
  * Spatial pruning: samples sorted by x0 on the host into 64 groups of
    512; each group computes over only NSEL=5 grid chunks of 128 points
    (those within RCUT of its x0 range) instead of all 20 (dropped
    Gaussian mass ~1e-3 relative). The host gathers per-group chunk
    operands so the device program stays static.
  * mm1 (K padded to KPAD): W^T(g,b) = exp(Pa_g^T @ Xa) -- the split-bf16
    augmented quadratic form gives the exponent directly. K is padded with
    zero rows because K=15 matmuls never trip the PE HAM activity monitor
    (the PE stays at 1.2 GHz); padded-K streams warm at 2.4 GHz.
  * mm2: py(m,b) += Ct_c(g,m)^T @ W_c^T(g,b); m = 64 columns
    [-mask*adj_x | -mask*adj_y | mask | pad] per time bin.
  * Epilogue on device is two cheap steps: r3 = py * onehot(tb) (DVE) and
    pr(4,b) = Bn^T @ r3 (one matmul summing each 20-bin segment). The
    final elementwise divide on (B,3) happens on the host after the
    gather, where it is trivial.
  * All inputs land in SBUF via one contiguous DMA per tensor (host
    pre-lays them out partition-major); junk warmup matmuls during the
    preload absorb the PE HAM ramp.
"""
import numpy as np
import ml_dtypes

import concourse.bass as bass
import concourse.tile as tile
from concourse import mybir, bacc
from concourse.bass_utils import run_bass_kernel_spmd

F32 = mybir.dt.float32
BF16 = mybir.dt.bfloat16
BF16_NP = ml_dtypes.bfloat16

B = 32768
G = 2500
GRID_SIZE = 50
NCHUNK = 20
NBINS = 20
NCORES = 8
BC = B // NCORES       # 4096 samples per core
BG = 512               # samples per group
NGRP = BC // BG        # 8 groups per core
NSEL = 5               # chunks kept per group
RCUT = 1.8             # x0 pruning radius
EPS = 1e-10
KPAD = 128             # contraction rows for mm1 (>=64; HAM warmth)
NWARM = 6              # junk warmup matmuls

_CACHE = {}


def _build_nc():
    nc = bacc.Bacc("TRN2", target_bir_lowering=False)
    xa_d = nc.dram_tensor("xa", [KPAD, NGRP * BG], BF16, kind="ExternalInput")
    pa_d = nc.dram_tensor("pa", [KPAD, NGRP * NSEL * 128], BF16,
                          kind="ExternalInput")
    ct_d = nc.dram_tensor("ct", [128, NGRP * NSEL * 64], BF16,
                          kind="ExternalInput")
    o3_d = nc.dram_tensor("o3", [64, NGRP * BG], BF16, kind="ExternalInput")
    bn_d = nc.dram_tensor("bn", [64, 4], BF16, kind="ExternalInput")
    o_d = nc.dram_tensor("o", [NGRP, 4, BG], F32, kind="ExternalOutput")

    with tile.TileContext(nc) as tc:
        with (
            tc.tile_pool(name="consts", bufs=1) as consts,
            tc.tile_pool(name="wt", bufs=2) as wtp,
            tc.tile_pool(name="ep", bufs=2) as ep,
            tc.tile_pool(name="pwa", bufs=1, space="PSUM") as pwa,
            tc.tile_pool(name="pwb", bufs=1, space="PSUM") as pwb,
            tc.tile_pool(name="py", bufs=1, space="PSUM") as pyp,
            tc.tile_pool(name="pr", bufs=1, space="PSUM") as prp,
        ):
            dum = consts.tile([128, BG], BF16)
            nc.vector.memset(dum[:], 0.5)
            pa_all = consts.tile([KPAD, NGRP, NSEL * 128], BF16)
            xa_all = consts.tile([KPAD, NGRP, BG], BF16)
            ct_all = consts.tile([128, NGRP, NSEL * 64], BF16)
            o3_all = consts.tile([64, NGRP, BG], BF16)
            bn_sb = consts.tile([64, 4], BF16)
            nc.sync.dma_start(out=bn_sb[:], in_=bn_d[:])
            CW, XW, OW = NSEL * 128, BG, BG
            for h in range(NGRP // 2):
                g0 = 2 * h
                nc.sync.dma_start(out=pa_all[:, g0:g0 + 2, :],
                                  in_=pa_d[:, g0 * CW:(g0 + 2) * CW])
                nc.sync.dma_start(out=xa_all[:, g0:g0 + 2, :],
                                  in_=xa_d[:, g0 * XW:(g0 + 2) * XW])
                nc.sync.dma_start(out=ct_all[:, g0:g0 + 2, :],
                                  in_=ct_d[:, g0 * NSEL * 64:(g0 + 2) * NSEL * 64])
                nc.sync.dma_start(out=o3_all[:, g0:g0 + 2, :],
                                  in_=o3_d[:, g0 * OW:(g0 + 2) * OW])

            def py_tile():
                return pyp.tile([64, BG], F32, tag="py", name="py")

            # warm the PE HAM while the preload DMAs land
            for i in range(NWARM):
                wu = py_tile()
                nc.tensor.matmul(wu[:], lhsT=dum[:, 0:64], rhs=dum[:],
                                 start=True, stop=True)

            def tail(state):
                (wt_a, wt_b), g = state
                py = py_tile()
                for c in range(NSEL):
                    nc.tensor.matmul(
                        py[:], lhsT=ct_all[:, g, c * 64:(c + 1) * 64],
                        rhs=(wt_a[:, c, :] if c < 2 else wt_b[:, c - 2, :]),
                        start=(c == 0), stop=(c == NSEL - 1))
                r3 = ep.tile([64, BG], BF16, tag="r3")
                nc.vector.tensor_mul(r3[:], py[:], o3_all[:, g])
                pr = prp.tile([4, BG], F32)
                nc.tensor.matmul(pr[:], lhsT=bn_sb[:], rhs=r3[:],
                                 start=True, stop=True)
                ot = ep.tile([4, BG], F32, tag="ot")
                nc.vector.tensor_copy(ot[:], pr[:])
                nc.sync.dma_start(out=o_d[g], in_=ot[:])

            state = None
            for g in range(NGRP):
                pw_a = pwa.tile([128, 2, BG], F32)
                for j in range(2):
                    nc.tensor.matmul(
                        pw_a[:, j, :],
                        lhsT=pa_all[:, g, j * 128:(j + 1) * 128],
                        rhs=xa_all[:, g], start=True, stop=True)
                wt_a = wtp.tile([128, 2, BG], BF16, tag="a")
                nc.scalar.activation(wt_a[:], pw_a[:],
                                     mybir.ActivationFunctionType.Exp)
                pw_b = pwb.tile([128, 3, BG], F32)
                for j in range(3):
                    c = 2 + j
                    nc.tensor.matmul(
                        pw_b[:, j, :],
                        lhsT=pa_all[:, g, c * 128:(c + 1) * 128],
                        rhs=xa_all[:, g], start=True, stop=True)
                wt_b = wtp.tile([128, 3, BG], BF16, tag="b")
                nc.scalar.activation(wt_b[:], pw_b[:],
                                     mybir.ActivationFunctionType.Exp)
                if state is not None:
                    tail(state)
                state = ((wt_a, wt_b), g)
            tail(state)
    nc.compile()
    return nc


_LIN = np.linspace(-8.0, 8.0, GRID_SIZE).astype(np.float32)
_CHUNK_LO = np.array([_LIN[(128 * c) // GRID_SIZE] for c in range(NCHUNK)])
_CHUNK_HI = np.array([_LIN[min((128 * c + 127) // GRID_SIZE, GRID_SIZE - 1)]
                      for c in range(NCHUNK)])


def _host_prep(t, x, grid_points, grid_adjoints, t_edges, grid_counts):
    t = np.asarray(t, np.float32).reshape(B)
    x = np.asarray(x, np.float32)
    gp = np.asarray(grid_points, np.float32)
    adj = np.asarray(grid_adjoints, np.float32)
    te = np.asarray(t_edges, np.float32)
    cnt = np.asarray(grid_counts)

    # global x0 sort -> 64 groups of 512 with narrow x0 bands
    perm = np.argsort(x[:, 0], kind='stable')
    xs = x[perm]
    ts = t[perm]

    # time-bin index (searchsorted-left semantics, clamped)
    tb = np.clip(np.searchsorted(te[1:-1], ts, side='left'), 0, NBINS - 1)

    # augmented X (KPAD, B): split-bf16 [xh; sqh; 1 | xl; sql; 0 | xh; sqh; 1]
    sq = xs * xs
    xh = xs.astype(BF16_NP)
    xl = (xs - xh.astype(np.float32)).astype(BF16_NP)
    sqh = sq.astype(BF16_NP)
    sql = (sq - sqh.astype(np.float32)).astype(BF16_NP)
    xa_full = np.zeros((KPAD, B), BF16_NP)
    for base, (c0, c1, one) in zip((0, 5, 10),
                                   ((xh, sqh, 1.0), (xl, sql, 0.0),
                                    (xh, sqh, 1.0))):
        xa_full[base + 0] = c0[:, 0]
        xa_full[base + 1] = c0[:, 1]
        xa_full[base + 2] = c1[:, 0]
        xa_full[base + 3] = c1[:, 1]
        xa_full[base + 4] = np.float32(one)

    # grid-side split-bf16 operand (15, GP)
    GP = NCHUNK * 128
    p5 = np.zeros((5, GP), np.float32)
    p5[0, :G] = 4.0 * gp[:, 0]
    p5[1, :G] = 4.0 * gp[:, 1]
    p5[2, :G] = -2.0
    p5[3, :G] = -2.0
    p5[4, :G] = -2.0 * (gp[:, 0] ** 2 + gp[:, 1] ** 2)
    p5[4, G:] = -1e30
    ph = p5.astype(BF16_NP)
    pl = (p5 - ph.astype(np.float32)).astype(BF16_NP)
    pa_full = np.zeros((KPAD, GP), BF16_NP)
    pa_full[0:5] = ph
    pa_full[5:10] = ph
    pa_full[10:15] = pl

    # ct (GP, 64): [-mask*adj_x | -mask*adj_y | mask | 0pad] per bin
    mask = (cnt > 0).astype(np.float32)                 # (20, G)
    ct_full = np.zeros((GP, 64), np.float32)
    ct_full[:G, 0:20] = -(mask * adj[:, :, 0]).T
    ct_full[:G, 20:40] = -(mask * adj[:, :, 1]).T
    ct_full[:G, 40:60] = mask.T
    ct_full = ct_full.astype(BF16_NP)

    # one-hot (64, B): rows tb, 20+tb, 40+tb are 1
    o3_full = np.zeros((64, B), BF16_NP)
    ar = np.arange(B)
    for dcol in range(3):
        o3_full[dcol * 20 + tb, ar] = np.float32(1.0)

    bn = np.zeros((64, 4), BF16_NP)
    for dcol in range(3):
        bn[dcol * 20:(dcol + 1) * 20, dcol] = np.float32(1.0)

    ngrp_total = B // BG
    sel_all = []
    for i in range(ngrp_total):
        seg = xs[i * BG:(i + 1) * BG, 0]
        a, b = seg.min(), seg.max()
        sel = [c for c in range(NCHUNK)
               if _CHUNK_HI[c] >= a - RCUT and _CHUNK_LO[c] <= b + RCUT]
        if not sel:
            sel = [-1]                   # no valid chunk: all-zero ct
        if len(sel) > NSEL:
            mid = 0.5 * (a + b)
            sel = sorted(sel,
                         key=lambda c: abs(0.5 * (_CHUNK_LO[c]
                                                  + _CHUNK_HI[c]) - mid))
            sel = sorted(sel[:NSEL])
        while len(sel) < NSEL:
            sel.append(sel[-1])          # duplicate pad; ct left zero
        sel_all.append(sel)

    in_maps = []
    for i in range(NCORES):
        xa_c = np.zeros((KPAD, NGRP, BG), BF16_NP)
        pa_c = np.zeros((KPAD, NGRP, NSEL * 128), BF16_NP)
        ct_c = np.zeros((128, NGRP, NSEL * 64), BF16_NP)
        o3_c = np.zeros((64, NGRP, BG), BF16_NP)
        for gl in range(NGRP):
            gi = i * NGRP + gl
            s0 = gi * BG
            xa_c[:, gl, :] = xa_full[:, s0:s0 + BG]
            o3_c[:, gl, :] = o3_full[:, s0:s0 + BG]
            sel = sel_all[gi]
            seen = set()
            for k, c in enumerate(sel):
                if c < 0:
                    pa_c[4, gl, k * 128:(k + 1) * 128] = np.float32(-1e30)
                    continue
                pa_c[:, gl, k * 128:(k + 1) * 128] = \
                    pa_full[:, c * 128:(c + 1) * 128]
                if c not in seen:
                    ct_c[:, gl, k * 64:(k + 1) * 64] = \
                        ct_full[c * 128:(c + 1) * 128, :]
                    seen.add(c)
        in_maps.append({
            "xa": np.ascontiguousarray(xa_c.reshape(KPAD, NGRP * BG)),
            "pa": np.ascontiguousarray(pa_c.reshape(KPAD, NGRP * NSEL * 128)),
            "ct": np.ascontiguousarray(ct_c.reshape(128, NGRP * NSEL * 64)),
            "o3": np.ascontiguousarray(o3_c.reshape(64, NGRP * BG)),
            "bn": bn,
        })
    return in_maps, perm


def kernel(t, x, grid_points, grid_adjoints, t_edges, grid_counts,
           trace=False, tmpdir=None):
    if "nc" not in _CACHE:
        _CACHE["nc"] = _build_nc()
    nc = _CACHE["nc"]
    in_maps, perm = _host_prep(t, x, grid_points, grid_adjoints, t_edges,
                               grid_counts)
    res = run_bass_kernel_spmd(nc, in_maps, core_ids=list(range(NCORES)),
                               trace=trace, tmpdir=tmpdir)
    _CACHE["last_result"] = res
    pys = np.concatenate([res.results[i]["o"].reshape(NGRP, 4, BG)
                          for i in range(NCORES)], axis=0)   # (64, 4, 512)
    pys = pys.transpose(0, 2, 1).reshape(B, 4)
    den = pys[:, 2] + np.float32(EPS)
    out_sorted = pys[:, 0:2] / den[:, None]
    out = np.empty((B, 2), np.float32)
    out[perm] = out_sorted.astype(np.float32)
    return out
